# revision 49
# baseline (speedup 1.0000x reference)
"""
Distributed Trainium2 (8 NeuronCore) kernel for a causal self-attention block:
    qkv = x @ W_proj + b_proj ; causal attention (eps-softmax, mask-before-scale)
    z = x + attn @ W_out + b_out ; out = layernorm(z) * gamma + beta

Sharding: core c computes batch b = c//4 with local heads 4r..4r+3 (r = c%4),
but OWNS output rows qb*512 + c*64 .. +64 of BOTH batches for every q-block qb.
After attention for q-block qb, the 8 cores exchange attention features with a
single 8-core AllToAll per qb (both head-pairs staged into one [8,2,128,64]
DRAM tile; slot p carries this core's 256 feature rows for p's 64 tokens).
Every slot carries real data because row ownership is split across both batch
groups. The receiver assembles the full 1024-feature attnT for its 128 owned
rows (64 per batch) and runs the FULL out-projection locally (same FLOPs as
the partial out-proj + ReduceScatter of y-partials, but 4x less collective
traffic and half the serial CC-core occupancy).

Scheduling rules learned from traces (engine queues are strict FIFO, so any
op waiting on a slow input head-of-line-blocks its whole engine):
  - out-proj/LN for q-block c-2 (not c-1!) is emitted mid-chunk c, so its
    collective finished a full chunk earlier and none of its PE/DVE/ACT ops
    ever wait (waiting ops would stall chunk c's exp/normalize streams and
    HAM-throttle the PE to 1.2 GHz).
  - attn@V matmuls trail the score matmuls by one key-block pair so the PE
    does not wait on the exp.
  - a2a read-DMAs are emitted at chunk end; Tile assigns collective-semaphore
    waits by cumulative count, so reads must not follow later collectives.
  - softmax reciprocal uses reciprocal_approx_fast (51 ULP, fine for a
    denominator in [0.02, 2500]); the stock reciprocal is ~6 Newton passes =
    3.3us on a [1,512] single-lane tile. Input must be staged to SBUF first
    (the custom DVE op mis-reads PSUM).

Other structure vs the naive version:
  - QKV (phase A) is interleaved with attention (phase B) per 512-token chunk,
    so the scalar engine's exp stream starts ~15us earlier and PE stays warm.
  - exp is batched: scores for 2 key-blocks accumulate into one [128,1024]
    2-bank PSUM tile -> one ACT exp instruction (halves ACT instruction count).
  - causal masking is done ON the tensor engine: a -30000 mask block is
    accumulated into the diagonal scores region via an identity-stationary
    matmul (start=False), replacing per-block DVE multiplies.
  - softmax eps add is dropped (denominator >= ~1e-2, eps=1e-9 is noise).
  - layernorm: sum(z^2) comes free from an ACT Square accum_out; centering and
    inv-std scaling fuse into one two-op tensor_scalar.

Compute dtype: bf16 matmul inputs, fp32 PSUM accumulation, fp32 softmax
denominators / layernorm statistics.

Layout rule (walrus limit): DMA descriptors carry at most 2 sync waits, and
Tile's wait assignment is not transitively minimal -- a DMA touching a recycled
SBUF region inherits waits on all 8 DMA queues. So every DMA-touched SBUF tile
lives in the never-recycled top-level pool; recycled phase pools hold
compute-engine-only tiles.
"""

import sys
import numpy as np

if "/opt/trn_rl_repo" not in sys.path:
    sys.path.insert(0, "/opt/trn_rl_repo")

B, T, D, H, HD = 2, 2048, 1024, 16, 64
NCORES = 8
ALL8 = [[0, 1, 2, 3, 4, 5, 6, 7]]
HL = 4            # heads per core
FL = HL * HD      # 256 local features
RS = 512          # output token rows per core
P = 128
QB = 512          # query block
KB = 128          # key block (partition axis)
NQB = T // QB     # 4
NDCH = D // P     # 8
OWN = 64          # rows owned per (batch, qb)

TRACE = False
LAST_RESULT = None
_GRAPH = None


def _build():
    import concourse.bass as bass
    import concourse.mybir as mybir
    from concourse import bacc, tile
    from concourse.masks import make_identity

    f32 = mybir.dt.float32
    bf16 = mybir.dt.bfloat16
    AF = mybir.ActivationFunctionType
    AX = mybir.AxisListType

    nc = bacc.Bacc(num_devices=NCORES)

    xT_ext = nc.declare_dram_parameter("xT", [D, T], bf16, isOutput=False)
    wq_ext = nc.declare_dram_parameter("wq", [D, FL], bf16, isOutput=False)
    wk_ext = nc.declare_dram_parameter("wk", [D, FL], bf16, isOutput=False)
    wv_ext = nc.declare_dram_parameter("wv", [D, FL], bf16, isOutput=False)
    bq_ext = nc.declare_dram_parameter("bq", [FL, 1], f32, isOutput=False)
    bk_ext = nc.declare_dram_parameter("bk", [FL, 1], f32, isOutput=False)
    bv_ext = nc.declare_dram_parameter("bv", [1, FL], f32, isOutput=False)
    wo_ext = nc.declare_dram_parameter("wo", [D, D], bf16, isOutput=False)
    bo_ext = nc.declare_dram_parameter("bo", [1, D], f32, isOutput=False)
    g_ext = nc.declare_dram_parameter("g", [1, D], f32, isOutput=False)
    bt_ext = nc.declare_dram_parameter("bt", [1, D], f32, isOutput=False)
    xr_ext = nc.declare_dram_parameter("xr", [RS, D], bf16, isOutput=False)
    out_ext = nc.declare_dram_parameter("out", [RS, D], f32, isOutput=True)

    with tile.TileContext(nc) as tc:
        with (
            tc.tile_pool(name="res", bufs=1) as res,
            tc.tile_pool(name="dram", bufs=1, space="DRAM") as dram,
        ):
            # ============ top-level (never-recycled) tiles ============
            # --- DMA-loaded inputs; issue order = critical path first:
            # biases -> xT chunk0 (split sync/gpsimd) -> wq -> wk -> wv
            # -> xT chunks 1..3 -> wo -> xr -> broadcast rows.
            # critical path: xT chunk0 + wq feed the very first matmul group.
            # Spread them over three DMA queues (sync/vector/scalar) and keep
            # the 16 tiny bias loads from clogging the sync queue head.
            xTt = [res.tile([P, T], bf16, tag=f"xTt{d}", name=f"xTt{d}") for d in range(NDCH)]
            for dch in range(NDCH):
                engq = nc.sync if dch % 2 == 0 else nc.gpsimd
                engq.dma_start(xTt[dch][:, 0:QB], xT_ext[dch * P:(dch + 1) * P, 0:QB])
            wqb = [res.tile([P, FL], bf16, tag=f"wqb{i}", name=f"wqb{i}") for i in range(NDCH)]
            wkb = [res.tile([P, FL], bf16, tag=f"wkb{i}", name=f"wkb{i}") for i in range(NDCH)]
            wvb = [res.tile([P, FL], bf16, tag=f"wvb{i}", name=f"wvb{i}") for i in range(NDCH)]
            for dch in range(NDCH):
                nc.scalar.dma_start(wqb[dch][:], wq_ext[dch * P:(dch + 1) * P, :])
            # broadcast-source rows early (tiny; gate chunk-0 setup)
            bvr = res.tile([1, FL], f32, tag="bvr", name="bvr")
            nc.scalar.dma_start(bvr[:], bv_ext[:, :])
            # biases coalesced into one DMA each (each dma_start costs ~600ns
            # of queue time regardless of size)
            bq2 = res.tile([P, 2], f32, tag="bq2", name="bq2")
            nc.sync.dma_start(bq2[:], bq_ext[:].rearrange("(i p) o -> p (i o)", p=P))
            bk2 = res.tile([P, 2], f32, tag="bk2", name="bk2")
            nc.sync.dma_start(bk2[:], bk_ext[:].rearrange("(i p) o -> p (i o)", p=P))

            for dch in range(NDCH):
                nc.scalar.dma_start(wvb[dch][:], wv_ext[dch * P:(dch + 1) * P, :])
            for dch in range(NDCH):
                nc.sync.dma_start(wkb[dch][:], wk_ext[dch * P:(dch + 1) * P, :])
            # --- chunk-0 compute setup, ahead of the bulk DMA on gpsimd ---
            vt = [res.tile([P, HL * (HD + 1)], bf16, tag=f"vt{i}", name=f"vt{i}")
                  for i in range(T // P)]
            for tt in range(T // P):
                v3 = vt[tt][:].rearrange("p (h e) -> p h e", e=HD + 1)
                nc.gpsimd.memset(v3[:, :, HD:HD + 1], 1.0)
            eps_t = res.tile([P, 1], f32, tag="eps", name="eps")
            nc.gpsimd.memset(eps_t[:], 1e-5)
            # additive causal mask for the diagonal 128x128 block:
            # maskM[k, q] = 0 if q >= k else -30000  (exp((S-30000)/8) == 0)
            maskM = res.tile([P, KB], bf16, tag="maskM", name="maskM")
            nc.gpsimd.memset(maskM[:], 0.0)
            nc.gpsimd.affine_select(
                out=maskM[:], in_=maskM[:],
                compare_op=mybir.AluOpType.is_ge, fill=-30000.0,
                base=0, channel_multiplier=-1,
                pattern=[[1, KB]],
            )
            ident = res.tile([P, P], bf16, tag="ident", name="ident")
            make_identity(nc, ident[:])
            bias_v = res.tile([P, FL], f32, tag="bias_v", name="bias_v")
            nc.gpsimd.partition_broadcast(bias_v[:], bvr[:])
            qTz = [res.tile([P, T], bf16, tag=f"qTz{i}", name=f"qTz{i}") for i in range(4)]
            for i in range(4):
                e = i % 2
                nc.vector.memset(qTz[i][(1 - e) * HD:(2 - e) * HD, :], 0.0)
            kT = [res.tile([P, T], bf16, tag=f"kT{i}", name=f"kT{i}") for i in range(2)]

            # --- bulk loads for later chunks ---
            for tch in range(1, NQB):
                for dch in range(NDCH):
                    engq = nc.sync if dch % 2 == 0 else nc.gpsimd
                    engq.dma_start(xTt[dch][:, tch * QB:(tch + 1) * QB],
                                   xT_ext[dch * P:(dch + 1) * P, tch * QB:(tch + 1) * QB])
            # full W_out, feature-major chunks
            wob = [res.tile([P, D], bf16, tag=f"wob{i}", name=f"wob{i}") for i in range(NDCH)]
            for dch in range(NDCH):
                nc.sync.dma_start(wob[dch][:], wo_ext[dch * P:(dch + 1) * P, :])
            # residual rows (bf16 cast); row layout: (qb, batch, 64)
            xrb = [res.tile([P, D], bf16, tag=f"xrb{i}", name=f"xrb{i}") for i in range(NQB)]
            for i in range(NQB):
                nc.sync.dma_start(xrb[i][:], xr_ext[i * P:(i + 1) * P, :])
            bor = res.tile([1, D], f32, tag="bor", name="bor")
            nc.gpsimd.dma_start(bor[:], bo_ext[:, :])
            gr = res.tile([1, D], f32, tag="gr", name="gr")
            nc.gpsimd.dma_start(gr[:], g_ext[:, :])
            btr = res.tile([1, D], f32, tag="btr", name="btr")
            nc.gpsimd.dma_start(btr[:], bt_ext[:, :])

            attnT = [res.tile([P, T], bf16, tag=f"attnT{i}", name=f"attnT{i}") for i in range(2)]
            # received attention features, double-buffered per (qb parity, hp):
            # [128 feat, (4 src, 2 batch-half, 64 tok)]
            allA = [[res.tile([P, 4 * P], bf16, tag=f"allA{par}{hp}", name=f"allA{par}{hp}")
                     for hp in range(2)] for par in range(2)]
            # layernorm broadcast tiles
            bo_bc = res.tile([P, D], f32, tag="bo_bc", name="bo_bc")
            nc.gpsimd.partition_broadcast(bo_bc[:], bor[:])
            g_bc = res.tile([P, D], f32, tag="g_bc", name="g_bc")
            nc.gpsimd.partition_broadcast(g_bc[:], gr[:])
            bt_bc = res.tile([P, D], f32, tag="bt_bc", name="bt_bc")
            nc.gpsimd.partition_broadcast(bt_bc[:], btr[:])
            bo_bcb = res.tile([P, D], bf16, tag="bo_bcb", name="bo_bcb")
            nc.vector.tensor_copy(bo_bcb[:], bo_bc[:])
            xrz = [res.tile([P, D], bf16, tag=f"xrz{i}", name=f"xrz{i}") for i in range(NQB)]
            for i in range(NQB):
                nc.vector.tensor_add(xrz[i][:], xrb[i][:], bo_bcb[:])
            ont = [res.tile([P, D], f32, tag=f"on{i}", name=f"on{i}") for i in range(NQB)]

            # per-(qb,hp) AllToAll bounce buffers: slot p = [128 feat, 64 tok]
            a2a_in = [dram.tile([NCORES, 2, P, OWN], bf16, name=f"a2a_in{q}")
                      for q in range(NQB)]
            a2a_out = [dram.tile([NCORES, 2, P, OWN], bf16, name=f"a2a_out{q}")
                       for q in range(NQB)]

            with (
                tc.tile_pool(name="psA", bufs=2, space="PSUM") as psA,
                tc.tile_pool(name="psS", bufs=2, space="PSUM") as psS,
                tc.tile_pool(name="psAt", bufs=1, space="PSUM") as psAt,
                tc.tile_pool(name="Ep", bufs=4) as Ep,
                tc.tile_pool(name="small", bufs=2) as small,
                tc.tile_pool(name="zp", bufs=2) as zp,
            ):
                def emit_qk(tch, ft):
                    # q^T, k^T for one head-pair (ft); attention for hp==ft
                    # depends only on this half, so it can start while the
                    # other half's projections run
                    for wb, is_q, bias in ((wqb, True, bq2), (wkb, False, bk2)):
                        if True:
                            ps = psA.tile([P, QB], f32, tag="psqk", name="psqk")
                            for dch in range(NDCH):
                                nc.tensor.matmul(ps[:], wb[dch][:, ft * P:(ft + 1) * P],
                                                 xTt[dch][:, tch * QB:(tch + 1) * QB],
                                                 start=(dch == 0), stop=(dch == NDCH - 1))
                            if is_q:
                                for e in range(2):
                                    nc.vector.tensor_scalar_add(
                                        qTz[ft * 2 + e][e * HD:(e + 1) * HD,
                                                        tch * QB:(tch + 1) * QB],
                                        ps[e * HD:(e + 1) * HD, :],
                                        bias[e * HD:(e + 1) * HD, ft:ft + 1])
                            else:
                                nc.vector.tensor_scalar_add(
                                    kT[ft][:, tch * QB:(tch + 1) * QB], ps[:],
                                    bias[:, ft:ft + 1])
                def emit_v(tch):
                    # v (token-major) + bias
                    for i in range(QB // P):
                        tt = tch * 4 + i
                        psv = psA.tile([P, QB], f32, tag="psqk", name="psv")
                        for dch in range(NDCH):
                            nc.tensor.matmul(psv[:, 0:FL], xTt[dch][:, tt * P:(tt + 1) * P],
                                             wvb[dch][:],
                                             start=(dch == 0), stop=(dch == NDCH - 1))
                        v3 = vt[tt][:].rearrange("p (h e) -> p h e", e=HD + 1)
                        nc.vector.tensor_tensor(
                            v3[:, :, 0:HD],
                            psv[:, 0:FL].rearrange("p (h d) -> p h d", d=HD),
                            bias_v[:].rearrange("p (h d) -> p h d", d=HD),
                            op=mybir.AluOpType.add)

                def emit_attn_hp(qb, hp, vhook=None):
                    nkb = (qb + 1) * (QB // KB)
                    if True:
                        pa = [psAt.tile([HD + 1, QB], f32, tag=f"psa{e}", name=f"psa{e}")
                              for e in range(2)]

                        def emit_av(i, Et):
                            # attn@V for pair i, consuming its exp'd Et tiles
                            for half in range(2):
                                kb = 2 * i + half
                                lo = max(0, (kb - qb * (QB // KB)) * KB)
                                mlo = 0 if kb == 0 else lo
                                base = half * QB
                                v3 = vt[kb][:].rearrange("p (h e) -> p h e", e=HD + 1)
                                for e in range(2):
                                    nc.tensor.matmul(
                                        pa[e][:, mlo:QB], v3[:, hp * 2 + e, :],
                                        Et[e][:, base + mlo:base + QB],
                                        start=(kb == 0), stop=(kb == nkb - 1))

                        prev = None
                        for i in range(nkb // 2):
                            # pair of key blocks (2i, 2i+1) -> one 2-bank PSUM
                            # tile + one exp per head e. Both halves' score
                            # matmuls span [plo:QB] so the exp window is fully
                            # written (masked cols are excluded from attn@V by
                            # the per-block window instead). attn@V trails one
                            # pair behind the scores so the PE never waits on
                            # the exp.
                            plo = max(0, (2 * i - qb * (QB // KB)) * KB)
                            ps = [psS.tile([P, 2 * QB], f32, tag="pss", name="pss")
                                  for _ in range(2)]
                            Et = [Ep.tile([P, 2 * QB], bf16, tag="E", name="E")
                                  for _ in range(2)]
                            for half in range(2):
                                kb = 2 * i + half
                                diag = kb - qb * (QB // KB) >= 0
                                base = half * QB
                                for e in range(2):
                                    nc.tensor.matmul(
                                        ps[e][:, base + plo:base + QB],
                                        kT[hp][:, kb * KB:(kb + 1) * KB],
                                        qTz[hp * 2 + e][:, qb * QB + plo:(qb + 1) * QB],
                                        start=True, stop=not diag)
                                if diag:
                                    lo = (kb - qb * (QB // KB)) * KB
                                    for e in range(2):
                                        nc.tensor.matmul(
                                            ps[e][:, base + lo:base + lo + KB],
                                            ident[:], maskM[:],
                                            start=False, stop=True)
                            for e in range(2):
                                nc.scalar.activation(Et[e][:, plo:2 * QB],
                                                     ps[e][:, plo:2 * QB],
                                                     AF.Exp, scale=0.125)
                            if i == 1 and vhook is not None:
                                # slot this chunk's v-projection behind the
                                # first score pairs so exp starts immediately
                                # at the chunk boundary
                                vhook()
                            if prev is not None:
                                emit_av(*prev)
                            prev = (i, Et)
                        emit_av(*prev)
                        for e in range(2):
                            den = small.tile([1, QB], f32, tag="den", name="den")
                            nc.vector.tensor_copy(den[:], pa[e][HD:HD + 1, :])
                            rec = small.tile([1, QB], f32, tag="rec", name="rec")
                            nc.vector.reciprocal_approx_fast(out=rec[:], in_=den[:])
                            bc = small.tile([HD, QB], f32, tag="bc", name="bc")
                            nc.gpsimd.partition_broadcast(bc[:], rec[:])
                            nc.vector.tensor_tensor(
                                attnT[hp][e * HD:(e + 1) * HD, qb * QB:(qb + 1) * QB],
                                pa[e][0:HD, :], bc[:], op=mybir.AluOpType.mult)
                        # ship this (qb, hp)'s features: slot p = my 128 feature
                        # rows for p's 64 tokens; one collective per qb after
                        # both head-pairs are staged
                        dst = a2a_in[qb][:, hp].rearrange("s p t -> p s t")
                        srcv = attnT[hp][:, qb * QB:(qb + 1) * QB].rearrange(
                            "p (s t) -> p s t", t=OWN)
                        nc.sync.dma_start(dst, srcv)
                        if hp == 1:
                            nc.gpsimd.collective_compute(
                                "AllToAll", mybir.AluOpType.bypass,
                                replica_groups=ALL8,
                                ins=[a2a_in[qb][:].opt()],
                                outs=[a2a_out[qb][:].opt()],
                            )

                def emit_a2a_read(qb):
                    aA = allA[qb % 2]
                    for hp in range(2):
                        for bh in range(2):
                            src = a2a_out[qb][bh * 4:(bh + 1) * 4, hp].rearrange(
                                "s p t -> p s t")
                            dstv = aA[hp][:].rearrange(
                                "p (s q2 t) -> p s q2 t", q2=2, t=OWN)[:, :, bh, :]
                            nc.sync.dma_start(dstv, src)

                def emit_out(qb):
                    aA = allA[qb % 2]
                    psy = psS.tile([P, 2 * QB], f32, tag="pss", name="psy")
                    # dt-major: bank 0 completes after 8 matmuls, so its
                    # residual-add runs under bank 1's matmuls
                    z = zp.tile([P, D], f32, tag="z", name="z")
                    for dt in range(2):
                        for fc in range(NDCH):
                            s, hp = fc // 2, fc % 2
                            nc.tensor.matmul(
                                psy[:, dt * QB:(dt + 1) * QB],
                                aA[hp][:, s * P:(s + 1) * P],
                                wob[fc][:, dt * QB:(dt + 1) * QB],
                                start=(fc == 0), stop=(fc == NDCH - 1))
                        nc.vector.tensor_tensor(
                            z[:, dt * QB:(dt + 1) * QB], psy[:, dt * QB:(dt + 1) * QB],
                            xrz[qb][:, dt * QB:(dt + 1) * QB], op=mybir.AluOpType.add)
                    s1 = small.tile([P, 1], f32, tag="s1", name="s1")
                    nc.vector.reduce_sum(s1[:], z[:], axis=AX.X)
                    mu = small.tile([P, 1], f32, tag="mu", name="mu")
                    nc.vector.tensor_scalar_mul(mu[:], s1[:], 1.0 / D)
                    # sum(z^2)/D via ACT Square accumulator (scale folded: (z/sqrt(D))^2)
                    scr = zp.tile([P, D], f32, tag="scr", name="scr")
                    zsq = small.tile([P, 1], f32, tag="zsq", name="zsq")
                    nc.scalar.activation(scr[:], z[:], AF.Square,
                                         scale=float(1.0 / np.sqrt(D)), accum_out=zsq[:])
                    mu2 = small.tile([P, 1], f32, tag="mu2", name="mu2")
                    nc.gpsimd.tensor_tensor(mu2[:], mu[:], mu[:], op=mybir.AluOpType.mult)
                    var = small.tile([P, 1], f32, tag="var", name="var")
                    nc.gpsimd.tensor_tensor(var[:], zsq[:], mu2[:],
                                            op=mybir.AluOpType.subtract)
                    # rsqrt(var + eps) = exp(-0.5 * ln(var + eps)): Log and Exp
                    # share one ACT table set, so no table switch away from the
                    # attention exps (Sqrt lives in a different set and cost
                    # ~2.6us of table reloads per LN)
                    lnv = small.tile([P, 1], f32, tag="lnv", name="lnv")
                    nc.scalar.activation(lnv[:], var[:], AF.Ln, bias=eps_t[:])
                    ri = small.tile([P, 1], f32, tag="ri", name="ri")
                    nc.scalar.activation(ri[:], lnv[:], AF.Exp, scale=-0.5)
                    on = ont[qb]
                    nc.vector.tensor_scalar(on[:], z[:], mu[:], ri[:],
                                            op0=mybir.AluOpType.subtract,
                                            op1=mybir.AluOpType.mult)
                    nc.vector.tensor_mul(on[:], on[:], g_bc[:])
                    nc.vector.tensor_add(on[:], on[:], bt_bc[:])
                    nc.sync.dma_start(out_ext[qb * P:(qb + 1) * P, :], on[:])

                for c in range(NQB):
                    emit_qk(c, 0)
                    emit_v(c)
                    emit_attn_hp(c, 0)
                    emit_qk(c, 1)
                    if c >= 2:
                        emit_out(c - 2)
                    emit_attn_hp(c, 1)
                    emit_a2a_read(c)
                emit_out(NQB - 2)
                emit_out(NQB - 1)

    nc.finalize()
    return nc


def kernel(**inputs):
    global _GRAPH, LAST_RESULT
    import ml_dtypes
    from concourse.bass_utils import run_bass_kernel_spmd
    bfnp = ml_dtypes.bfloat16

    x = np.ascontiguousarray(inputs["x"], dtype=np.float32)
    W_proj = np.ascontiguousarray(inputs["W_proj"], dtype=np.float32)
    b_proj = np.ascontiguousarray(inputs["b_proj"], dtype=np.float32)
    W_out = np.ascontiguousarray(inputs["W_out"], dtype=np.float32)
    b_out = np.ascontiguousarray(inputs["b_out"], dtype=np.float32)
    ln_g = np.ascontiguousarray(inputs["ln_gamma"], dtype=np.float32)
    ln_b = np.ascontiguousarray(inputs["ln_beta"], dtype=np.float32)

    if _GRAPH is None:
        _GRAPH = _build()
    nc = _GRAPH

    wo_full = np.ascontiguousarray(W_out).astype(bfnp)
    in_maps = []
    for c in range(NCORES):
        b, r = c // 4, c % 4
        cs = slice(r * FL, (r + 1) * FL)
        xr_rows = np.concatenate(
            [x[bb, qq * QB + c * OWN: qq * QB + (c + 1) * OWN]
             for qq in range(NQB) for bb in range(B)], axis=0)
        in_maps.append({
            "xT": np.ascontiguousarray(x[b].T).astype(bfnp),
            "wq": np.ascontiguousarray(W_proj[:, cs]).astype(bfnp),
            "wk": np.ascontiguousarray(W_proj[:, D + r * FL: D + (r + 1) * FL]).astype(bfnp),
            "wv": np.ascontiguousarray(W_proj[:, 2 * D + r * FL: 2 * D + (r + 1) * FL]).astype(bfnp),
            "bq": np.ascontiguousarray(b_proj[cs].reshape(FL, 1)),
            "bk": np.ascontiguousarray(b_proj[D + r * FL: D + (r + 1) * FL].reshape(FL, 1)),
            "bv": np.ascontiguousarray(b_proj[2 * D + r * FL: 2 * D + (r + 1) * FL].reshape(1, FL)),
            "wo": wo_full,
            "bo": b_out.reshape(1, D),
            "g": ln_g.reshape(1, D),
            "bt": ln_b.reshape(1, D),
            "xr": np.ascontiguousarray(xr_rows).astype(bfnp),
        })

    res = run_bass_kernel_spmd(nc, in_maps, core_ids=list(range(NCORES)),
                               trace=TRACE)
    LAST_RESULT = res

    out = np.empty((B, T, D), np.float32)
    for c in range(NCORES):
        oc = res.results[c]["out"]
        for qq in range(NQB):
            for bb in range(B):
                out[bb, qq * QB + c * OWN: qq * QB + (c + 1) * OWN, :] = \
                    oc[qq * P + bb * OWN: qq * P + (bb + 1) * OWN]
    return out


# revision 52
# speedup vs baseline: 1.0378x; 1.0378x over previous
"""
Distributed Trainium2 (8 NeuronCore) kernel for a causal self-attention block:
    qkv = x @ W_proj + b_proj ; causal attention (eps-softmax, mask-before-scale)
    z = x + attn @ W_out + b_out ; out = layernorm(z) * gamma + beta

Sharding: core c computes batch b = c//4 with local heads 4r..4r+3 (r = c%4),
but OWNS output rows qb*512 + c*64 .. +64 of BOTH batches for every q-block qb.
After attention for q-block qb, the 8 cores exchange attention features with a
single 8-core AllToAll per qb (both head-pairs staged into one [8,2,128,64]
DRAM tile; slot p carries this core's 256 feature rows for p's 64 tokens).
Every slot carries real data because row ownership is split across both batch
groups. The receiver assembles the full 1024-feature attnT for its 128 owned
rows (64 per batch) and runs the FULL out-projection locally (same FLOPs as
the partial out-proj + ReduceScatter of y-partials, but 4x less collective
traffic and half the serial CC-core occupancy).

Scheduling rules learned from traces (engine queues are strict FIFO, so any
op waiting on a slow input head-of-line-blocks its whole engine):
  - out-proj/LN for q-block c-2 (not c-1!) is emitted mid-chunk c, so its
    collective finished a full chunk earlier and none of its PE/DVE/ACT ops
    ever wait (waiting ops would stall chunk c's exp/normalize streams and
    HAM-throttle the PE to 1.2 GHz).
  - attn@V matmuls trail the score matmuls by one key-block pair so the PE
    does not wait on the exp.
  - a2a read-DMAs are emitted at chunk end; Tile assigns collective-semaphore
    waits by cumulative count, so reads must not follow later collectives.
  - softmax reciprocal uses reciprocal_approx_fast (51 ULP, fine for a
    denominator in [0.02, 2500]); the stock reciprocal is ~6 Newton passes =
    3.3us on a [1,512] single-lane tile. Input must be staged to SBUF first
    (the custom DVE op mis-reads PSUM).

Other structure vs the naive version:
  - QKV (phase A) is interleaved with attention (phase B) per 512-token chunk,
    so the scalar engine's exp stream starts ~15us earlier and PE stays warm.
  - exp is batched: scores for 2 key-blocks accumulate into one [128,1024]
    2-bank PSUM tile -> one ACT exp instruction (halves ACT instruction count).
  - causal masking is done ON the tensor engine: a -30000 mask block is
    accumulated into the diagonal scores region via an identity-stationary
    matmul (start=False), replacing per-block DVE multiplies.
  - softmax eps add is dropped (denominator >= ~1e-2, eps=1e-9 is noise).
  - layernorm: sum(z^2) comes free from an ACT Square accum_out; centering and
    inv-std scaling fuse into one two-op tensor_scalar.

Compute dtype: bf16 matmul inputs, fp32 PSUM accumulation, fp32 softmax
denominators / layernorm statistics.

Layout rule (walrus limit): DMA descriptors carry at most 2 sync waits, and
Tile's wait assignment is not transitively minimal -- a DMA touching a recycled
SBUF region inherits waits on all 8 DMA queues. So every DMA-touched SBUF tile
lives in the never-recycled top-level pool; recycled phase pools hold
compute-engine-only tiles.
"""

import sys
import numpy as np

if "/opt/trn_rl_repo" not in sys.path:
    sys.path.insert(0, "/opt/trn_rl_repo")

B, T, D, H, HD = 2, 2048, 1024, 16, 64
NCORES = 8
ALL8 = [[0, 1, 2, 3, 4, 5, 6, 7]]
HL = 4            # heads per core
FL = HL * HD      # 256 local features
RS = 512          # output token rows per core
P = 128
QB = 512          # query block
KB = 128          # key block (partition axis)
NQB = T // QB     # 4
NDCH = D // P     # 8
OWN = 64          # rows owned per (batch, qb)

TRACE = False
LAST_RESULT = None
_GRAPH = None


def _build():
    import concourse.bass as bass
    import concourse.mybir as mybir
    from concourse import bacc, tile
    from concourse.masks import make_identity

    f32 = mybir.dt.float32
    bf16 = mybir.dt.bfloat16
    AF = mybir.ActivationFunctionType
    AX = mybir.AxisListType

    nc = bacc.Bacc(num_devices=NCORES)

    xT_ext = nc.declare_dram_parameter("xT", [D, T], bf16, isOutput=False)
    wq_ext = nc.declare_dram_parameter("wq", [D, FL], bf16, isOutput=False)
    wk_ext = nc.declare_dram_parameter("wk", [D, FL], bf16, isOutput=False)
    wv_ext = nc.declare_dram_parameter("wv", [D, FL], bf16, isOutput=False)
    bq_ext = nc.declare_dram_parameter("bq", [FL, 1], f32, isOutput=False)
    bk_ext = nc.declare_dram_parameter("bk", [FL, 1], f32, isOutput=False)
    bv_ext = nc.declare_dram_parameter("bv", [1, FL], f32, isOutput=False)
    wo_ext = nc.declare_dram_parameter("wo", [D, D], bf16, isOutput=False)
    bo_ext = nc.declare_dram_parameter("bo", [1, D], f32, isOutput=False)
    g_ext = nc.declare_dram_parameter("g", [1, D], f32, isOutput=False)
    bt_ext = nc.declare_dram_parameter("bt", [1, D], f32, isOutput=False)
    xr_ext = nc.declare_dram_parameter("xr", [RS, D], bf16, isOutput=False)
    out_ext = nc.declare_dram_parameter("out", [RS, D], f32, isOutput=True)

    with tile.TileContext(nc) as tc:
        with (
            tc.tile_pool(name="res", bufs=1) as res,
            tc.tile_pool(name="dram", bufs=1, space="DRAM") as dram,
        ):
            # ============ top-level (never-recycled) tiles ============
            # --- DMA-loaded inputs; issue order = critical path first:
            # biases -> xT chunk0 (split sync/gpsimd) -> wq -> wk -> wv
            # -> xT chunks 1..3 -> wo -> xr -> broadcast rows.
            # critical path: xT chunk0 + wq feed the very first matmul group.
            # Spread them over three DMA queues (sync/vector/scalar) and keep
            # the 16 tiny bias loads from clogging the sync queue head.
            xTt = [res.tile([P, T], bf16, tag=f"xTt{d}", name=f"xTt{d}") for d in range(NDCH)]
            for dch in range(NDCH):
                engq = nc.sync if dch % 2 == 0 else nc.gpsimd
                engq.dma_start(xTt[dch][:, 0:QB], xT_ext[dch * P:(dch + 1) * P, 0:QB])
            wqb = [res.tile([P, FL], bf16, tag=f"wqb{i}", name=f"wqb{i}") for i in range(NDCH)]
            wkb = [res.tile([P, FL], bf16, tag=f"wkb{i}", name=f"wkb{i}") for i in range(NDCH)]
            wvb = [res.tile([P, FL], bf16, tag=f"wvb{i}", name=f"wvb{i}") for i in range(NDCH)]
            for dch in range(NDCH):
                nc.scalar.dma_start(wqb[dch][:], wq_ext[dch * P:(dch + 1) * P, :])
            # broadcast-source rows early (tiny; gate chunk-0 setup)
            bvr = res.tile([1, FL], f32, tag="bvr", name="bvr")
            nc.scalar.dma_start(bvr[:], bv_ext[:, :])
            # biases coalesced into one DMA each (each dma_start costs ~600ns
            # of queue time regardless of size)
            bq2 = res.tile([P, 2], f32, tag="bq2", name="bq2")
            nc.sync.dma_start(bq2[:], bq_ext[:].rearrange("(i p) o -> p (i o)", p=P))
            bk2 = res.tile([P, 2], f32, tag="bk2", name="bk2")
            nc.sync.dma_start(bk2[:], bk_ext[:].rearrange("(i p) o -> p (i o)", p=P))

            for dch in range(NDCH):
                nc.scalar.dma_start(wvb[dch][:], wv_ext[dch * P:(dch + 1) * P, :])
            for dch in range(NDCH):
                nc.sync.dma_start(wkb[dch][:], wk_ext[dch * P:(dch + 1) * P, :])
            # --- chunk-0 compute setup, ahead of the bulk DMA on gpsimd ---
            vt = [res.tile([P, HL * (HD + 1)], bf16, tag=f"vt{i}", name=f"vt{i}")
                  for i in range(T // P)]
            for tt in range(T // P):
                v3 = vt[tt][:].rearrange("p (h e) -> p h e", e=HD + 1)
                nc.gpsimd.memset(v3[:, :, HD:HD + 1], 1.0)
            eps_t = res.tile([P, 1], f32, tag="eps", name="eps")
            nc.gpsimd.memset(eps_t[:], 1e-5)
            # additive causal mask for the diagonal 128x128 block:
            # maskM[k, q] = 0 if q >= k else -30000  (exp((S-30000)/8) == 0)
            maskM = res.tile([P, KB], bf16, tag="maskM", name="maskM")
            nc.gpsimd.memset(maskM[:], 0.0)
            nc.gpsimd.affine_select(
                out=maskM[:], in_=maskM[:],
                compare_op=mybir.AluOpType.is_ge, fill=-30000.0,
                base=0, channel_multiplier=-1,
                pattern=[[1, KB]],
            )
            ident = res.tile([P, P], bf16, tag="ident", name="ident")
            make_identity(nc, ident[:])
            bias_v = res.tile([P, FL], f32, tag="bias_v", name="bias_v")
            nc.gpsimd.partition_broadcast(bias_v[:], bvr[:])
            qTz = [res.tile([P, T], bf16, tag=f"qTz{i}", name=f"qTz{i}") for i in range(4)]
            for i in range(4):
                e = i % 2
                nc.vector.memset(qTz[i][(1 - e) * HD:(2 - e) * HD, :], 0.0)
            kT = [res.tile([P, T], bf16, tag=f"kT{i}", name=f"kT{i}") for i in range(2)]

            # --- bulk loads for later chunks ---
            for tch in range(1, NQB):
                for dch in range(NDCH):
                    engq = nc.sync if dch % 2 == 0 else nc.gpsimd
                    engq.dma_start(xTt[dch][:, tch * QB:(tch + 1) * QB],
                                   xT_ext[dch * P:(dch + 1) * P, tch * QB:(tch + 1) * QB])
            # full W_out, feature-major chunks
            wob = [res.tile([P, D], bf16, tag=f"wob{i}", name=f"wob{i}") for i in range(NDCH)]
            for dch in range(NDCH):
                nc.sync.dma_start(wob[dch][:], wo_ext[dch * P:(dch + 1) * P, :])
            # residual rows (bf16 cast); row layout: (qb, batch, 64)
            xrb = [res.tile([P, D], bf16, tag=f"xrb{i}", name=f"xrb{i}") for i in range(NQB)]
            for i in range(NQB):
                nc.sync.dma_start(xrb[i][:], xr_ext[i * P:(i + 1) * P, :])
            bor = res.tile([1, D], f32, tag="bor", name="bor")
            nc.gpsimd.dma_start(bor[:], bo_ext[:, :])
            gr = res.tile([1, D], f32, tag="gr", name="gr")
            nc.gpsimd.dma_start(gr[:], g_ext[:, :])
            btr = res.tile([1, D], f32, tag="btr", name="btr")
            nc.gpsimd.dma_start(btr[:], bt_ext[:, :])

            attnT = [res.tile([P, T], bf16, tag=f"attnT{i}", name=f"attnT{i}") for i in range(2)]
            # received attention features, double-buffered per (qb parity, hp):
            # [128 feat, (4 src, 2 batch-half, 64 tok)]
            allA = [[res.tile([P, 4 * P], bf16, tag=f"allA{par}{hp}", name=f"allA{par}{hp}")
                     for hp in range(2)] for par in range(2)]
            # layernorm broadcast tiles
            bo_bc = res.tile([P, D], f32, tag="bo_bc", name="bo_bc")
            nc.gpsimd.partition_broadcast(bo_bc[:], bor[:])
            g_bc = res.tile([P, D], f32, tag="g_bc", name="g_bc")
            nc.gpsimd.partition_broadcast(g_bc[:], gr[:])
            bt_bc = res.tile([P, D], f32, tag="bt_bc", name="bt_bc")
            nc.gpsimd.partition_broadcast(bt_bc[:], btr[:])
            bo_bcb = res.tile([P, D], bf16, tag="bo_bcb", name="bo_bcb")
            nc.vector.tensor_copy(bo_bcb[:], bo_bc[:])
            xrz = [res.tile([P, D], bf16, tag=f"xrz{i}", name=f"xrz{i}") for i in range(NQB)]
            for i in range(NQB):
                nc.vector.tensor_add(xrz[i][:], xrb[i][:], bo_bcb[:])
            ont = [res.tile([P, D], f32, tag=f"on{i}", name=f"on{i}") for i in range(NQB)]

            # per-(qb,hp) AllToAll bounce buffers: slot p = [128 feat, 64 tok]
            a2a_in = [dram.tile([NCORES, 2, P, OWN], bf16, name=f"a2a_in{q}")
                      for q in range(NQB)]
            a2a_out = [dram.tile([NCORES, 2, P, OWN], bf16, name=f"a2a_out{q}")
                       for q in range(NQB)]

            with (
                tc.tile_pool(name="psA", bufs=2, space="PSUM") as psA,
                tc.tile_pool(name="psS", bufs=2, space="PSUM") as psS,
                tc.tile_pool(name="psAt", bufs=1, space="PSUM") as psAt,
                tc.tile_pool(name="Ep", bufs=4) as Ep,
                tc.tile_pool(name="small", bufs=2) as small,
                tc.tile_pool(name="zp", bufs=2) as zp,
            ):
                def emit_qk(tch, ft):
                    # q^T, k^T for one head-pair (ft); attention for hp==ft
                    # depends only on this half, so it can start while the
                    # other half's projections run
                    for wb, is_q, bias in ((wqb, True, bq2), (wkb, False, bk2)):
                        if True:
                            ps = psA.tile([P, QB], f32, tag="psqk", name="psqk")
                            for dch in range(NDCH):
                                nc.tensor.matmul(ps[:], wb[dch][:, ft * P:(ft + 1) * P],
                                                 xTt[dch][:, tch * QB:(tch + 1) * QB],
                                                 start=(dch == 0), stop=(dch == NDCH - 1))
                            if is_q:
                                for e in range(2):
                                    nc.vector.tensor_scalar_add(
                                        qTz[ft * 2 + e][e * HD:(e + 1) * HD,
                                                        tch * QB:(tch + 1) * QB],
                                        ps[e * HD:(e + 1) * HD, :],
                                        bias[e * HD:(e + 1) * HD, ft:ft + 1])
                            else:
                                nc.vector.tensor_scalar_add(
                                    kT[ft][:, tch * QB:(tch + 1) * QB], ps[:],
                                    bias[:, ft:ft + 1])
                def emit_v(tch):
                    # v (token-major) + bias
                    for i in range(QB // P):
                        tt = tch * 4 + i
                        psv = psA.tile([P, QB], f32, tag="psqk", name="psv")
                        for dch in range(NDCH):
                            nc.tensor.matmul(psv[:, 0:FL], xTt[dch][:, tt * P:(tt + 1) * P],
                                             wvb[dch][:],
                                             start=(dch == 0), stop=(dch == NDCH - 1))
                        v3 = vt[tt][:].rearrange("p (h e) -> p h e", e=HD + 1)
                        nc.vector.tensor_tensor(
                            v3[:, :, 0:HD],
                            psv[:, 0:FL].rearrange("p (h d) -> p h d", d=HD),
                            bias_v[:].rearrange("p (h d) -> p h d", d=HD),
                            op=mybir.AluOpType.add)

                def emit_attn_hp(qb, hp, vhook=None):
                    nkb = (qb + 1) * (QB // KB)
                    if True:
                        pa = [psAt.tile([HD + 1, QB], f32, tag=f"psa{e}", name=f"psa{e}")
                              for e in range(2)]

                        def emit_av(i, Et):
                            # attn@V for pair i, consuming its exp'd Et tiles
                            for half in range(2):
                                kb = 2 * i + half
                                lo = max(0, (kb - qb * (QB // KB)) * KB)
                                mlo = 0 if kb == 0 else lo
                                base = half * QB
                                v3 = vt[kb][:].rearrange("p (h e) -> p h e", e=HD + 1)
                                for e in range(2):
                                    nc.tensor.matmul(
                                        pa[e][:, mlo:QB], v3[:, hp * 2 + e, :],
                                        Et[e][:, base + mlo:base + QB],
                                        start=(kb == 0), stop=(kb == nkb - 1))

                        prev = None
                        for i in range(nkb // 2):
                            # pair of key blocks (2i, 2i+1) -> one 2-bank PSUM
                            # tile + one exp per head e. Both halves' score
                            # matmuls span [plo:QB] so the exp window is fully
                            # written (masked cols are excluded from attn@V by
                            # the per-block window instead). attn@V trails one
                            # pair behind the scores so the PE never waits on
                            # the exp.
                            plo = max(0, (2 * i - qb * (QB // KB)) * KB)
                            ps = [psS.tile([P, 2 * QB], f32, tag="pss", name="pss")
                                  for _ in range(2)]
                            Et = [Ep.tile([P, 2 * QB], bf16, tag="E", name="E")
                                  for _ in range(2)]
                            for half in range(2):
                                kb = 2 * i + half
                                diag = kb - qb * (QB // KB) >= 0
                                base = half * QB
                                for e in range(2):
                                    nc.tensor.matmul(
                                        ps[e][:, base + plo:base + QB],
                                        kT[hp][:, kb * KB:(kb + 1) * KB],
                                        qTz[hp * 2 + e][:, qb * QB + plo:(qb + 1) * QB],
                                        start=True, stop=not diag)
                                if diag:
                                    lo = (kb - qb * (QB // KB)) * KB
                                    for e in range(2):
                                        nc.tensor.matmul(
                                            ps[e][:, base + lo:base + lo + KB],
                                            ident[:], maskM[:],
                                            start=False, stop=True)
                            for e in range(2):
                                nc.scalar.activation(Et[e][:, plo:2 * QB],
                                                     ps[e][:, plo:2 * QB],
                                                     AF.Exp, scale=0.125)
                            if i == 1 and vhook is not None:
                                # slot this chunk's v-projection behind the
                                # first score pairs so exp starts immediately
                                # at the chunk boundary
                                vhook()
                            if prev is not None:
                                emit_av(*prev)
                            prev = (i, Et)
                        emit_av(*prev)
                        for e in range(2):
                            den = small.tile([1, QB], f32, tag="den", name="den")
                            nc.vector.tensor_copy(den[:], pa[e][HD:HD + 1, :])
                            rec = small.tile([1, QB], f32, tag="rec", name="rec")
                            nc.vector.reciprocal_approx_fast(out=rec[:], in_=den[:])
                            bc = small.tile([HD, QB], f32, tag="bc", name="bc")
                            nc.gpsimd.partition_broadcast(bc[:], rec[:])
                            nc.vector.tensor_tensor(
                                attnT[hp][e * HD:(e + 1) * HD, qb * QB:(qb + 1) * QB],
                                pa[e][0:HD, :], bc[:], op=mybir.AluOpType.mult)
                        # ship this (qb, hp)'s features: slot p = my 128 feature
                        # rows for p's 64 tokens; one collective per qb after
                        # both head-pairs are staged
                        dst = a2a_in[qb][:, hp].rearrange("s p t -> p s t")
                        srcv = attnT[hp][:, qb * QB:(qb + 1) * QB].rearrange(
                            "p (s t) -> p s t", t=OWN)
                        nc.sync.dma_start(dst, srcv)
                        if hp == 1:
                            nc.gpsimd.collective_compute(
                                "AllToAll", mybir.AluOpType.bypass,
                                replica_groups=ALL8,
                                ins=[a2a_in[qb][:].opt()],
                                outs=[a2a_out[qb][:].opt()],
                            )

                def emit_a2a_read(qb):
                    aA = allA[qb % 2]
                    for hp in range(2):
                        for bh in range(2):
                            src = a2a_out[qb][bh * 4:(bh + 1) * 4, hp].rearrange(
                                "s p t -> p s t")
                            dstv = aA[hp][:].rearrange(
                                "p (s q2 t) -> p s q2 t", q2=2, t=OWN)[:, :, bh, :]
                            nc.sync.dma_start(dstv, src)

                def emit_out(qb):
                    aA = allA[qb % 2]
                    psy = psS.tile([P, 2 * QB], f32, tag="pss", name="psy")
                    # dt-major: bank 0 completes after 8 matmuls, so its
                    # residual-add runs under bank 1's matmuls
                    z = zp.tile([P, D], f32, tag="z", name="z")
                    for dt in range(2):
                        # hp0 feature chunks first: on the last q-block they
                        # can run while hp1's exchange is still in flight
                        for n, fc in enumerate((0, 2, 4, 6, 1, 3, 5, 7)):
                            s, hp = fc // 2, fc % 2
                            nc.tensor.matmul(
                                psy[:, dt * QB:(dt + 1) * QB],
                                aA[hp][:, s * P:(s + 1) * P],
                                wob[fc][:, dt * QB:(dt + 1) * QB],
                                start=(n == 0), stop=(n == NDCH - 1))
                        nc.vector.tensor_tensor(
                            z[:, dt * QB:(dt + 1) * QB], psy[:, dt * QB:(dt + 1) * QB],
                            xrz[qb][:, dt * QB:(dt + 1) * QB], op=mybir.AluOpType.add)
                    s1 = small.tile([P, 1], f32, tag="s1", name="s1")
                    nc.vector.reduce_sum(s1[:], z[:], axis=AX.X)
                    mu = small.tile([P, 1], f32, tag="mu", name="mu")
                    nc.vector.tensor_scalar_mul(mu[:], s1[:], 1.0 / D)
                    # sum(z^2)/D via ACT Square accumulator (scale folded: (z/sqrt(D))^2)
                    scr = zp.tile([P, D], f32, tag="scr", name="scr")
                    zsq = small.tile([P, 1], f32, tag="zsq", name="zsq")
                    nc.scalar.activation(scr[:], z[:], AF.Square,
                                         scale=float(1.0 / np.sqrt(D)), accum_out=zsq[:])
                    mu2 = small.tile([P, 1], f32, tag="mu2", name="mu2")
                    nc.gpsimd.tensor_tensor(mu2[:], mu[:], mu[:], op=mybir.AluOpType.mult)
                    var = small.tile([P, 1], f32, tag="var", name="var")
                    nc.gpsimd.tensor_tensor(var[:], zsq[:], mu2[:],
                                            op=mybir.AluOpType.subtract)
                    # rsqrt(var + eps) = exp(-0.5 * ln(var + eps)): Log and Exp
                    # share one ACT table set, so no table switch away from the
                    # attention exps (Sqrt lives in a different set and cost
                    # ~2.6us of table reloads per LN)
                    lnv = small.tile([P, 1], f32, tag="lnv", name="lnv")
                    nc.scalar.activation(lnv[:], var[:], AF.Ln, bias=eps_t[:])
                    ri = small.tile([P, 1], f32, tag="ri", name="ri")
                    nc.scalar.activation(ri[:], lnv[:], AF.Exp, scale=-0.5)
                    on = ont[qb]
                    nc.vector.tensor_scalar(on[:], z[:], mu[:], ri[:],
                                            op0=mybir.AluOpType.subtract,
                                            op1=mybir.AluOpType.mult)
                    nc.vector.tensor_mul(on[:], on[:], g_bc[:])
                    nc.vector.tensor_add(on[:], on[:], bt_bc[:])
                    nc.sync.dma_start(out_ext[qb * P:(qb + 1) * P, :], on[:])

                for c in range(NQB):
                    emit_qk(c, 0)
                    emit_v(c)
                    emit_attn_hp(c, 0)
                    emit_qk(c, 1)
                    if c >= 2:
                        emit_out(c - 2)
                    emit_attn_hp(c, 1)
                    emit_a2a_read(c)
                emit_out(NQB - 2)
                emit_out(NQB - 1)

    nc.finalize()
    return nc


def kernel(**inputs):
    global _GRAPH, LAST_RESULT
    import ml_dtypes
    from concourse.bass_utils import run_bass_kernel_spmd
    bfnp = ml_dtypes.bfloat16

    x = np.ascontiguousarray(inputs["x"], dtype=np.float32)
    W_proj = np.ascontiguousarray(inputs["W_proj"], dtype=np.float32)
    b_proj = np.ascontiguousarray(inputs["b_proj"], dtype=np.float32)
    W_out = np.ascontiguousarray(inputs["W_out"], dtype=np.float32)
    b_out = np.ascontiguousarray(inputs["b_out"], dtype=np.float32)
    ln_g = np.ascontiguousarray(inputs["ln_gamma"], dtype=np.float32)
    ln_b = np.ascontiguousarray(inputs["ln_beta"], dtype=np.float32)

    if _GRAPH is None:
        _GRAPH = _build()
    nc = _GRAPH

    wo_full = np.ascontiguousarray(W_out).astype(bfnp)
    in_maps = []
    for c in range(NCORES):
        b, r = c // 4, c % 4
        cs = slice(r * FL, (r + 1) * FL)
        xr_rows = np.concatenate(
            [x[bb, qq * QB + c * OWN: qq * QB + (c + 1) * OWN]
             for qq in range(NQB) for bb in range(B)], axis=0)
        in_maps.append({
            "xT": np.ascontiguousarray(x[b].T).astype(bfnp),
            "wq": np.ascontiguousarray(W_proj[:, cs]).astype(bfnp),
            "wk": np.ascontiguousarray(W_proj[:, D + r * FL: D + (r + 1) * FL]).astype(bfnp),
            "wv": np.ascontiguousarray(W_proj[:, 2 * D + r * FL: 2 * D + (r + 1) * FL]).astype(bfnp),
            "bq": np.ascontiguousarray(b_proj[cs].reshape(FL, 1)),
            "bk": np.ascontiguousarray(b_proj[D + r * FL: D + (r + 1) * FL].reshape(FL, 1)),
            "bv": np.ascontiguousarray(b_proj[2 * D + r * FL: 2 * D + (r + 1) * FL].reshape(1, FL)),
            "wo": wo_full,
            "bo": b_out.reshape(1, D),
            "g": ln_g.reshape(1, D),
            "bt": ln_b.reshape(1, D),
            "xr": np.ascontiguousarray(xr_rows).astype(bfnp),
        })

    res = run_bass_kernel_spmd(nc, in_maps, core_ids=list(range(NCORES)),
                               trace=TRACE)
    LAST_RESULT = res

    out = np.empty((B, T, D), np.float32)
    for c in range(NCORES):
        oc = res.results[c]["out"]
        for qq in range(NQB):
            for bb in range(B):
                out[bb, qq * QB + c * OWN: qq * QB + (c + 1) * OWN, :] = \
                    oc[qq * P + bb * OWN: qq * P + (bb + 1) * OWN]
    return out


# revision 56
# speedup vs baseline: 1.0401x; 1.0023x over previous
"""
Distributed Trainium2 (8 NeuronCore) kernel for a causal self-attention block:
    qkv = x @ W_proj + b_proj ; causal attention (eps-softmax, mask-before-scale)
    z = x + attn @ W_out + b_out ; out = layernorm(z) * gamma + beta

Sharding: core c computes batch b = c//4 with local heads 4r..4r+3 (r = c%4),
but OWNS output rows qb*512 + c*64 .. +64 of BOTH batches for every q-block qb.
After attention for q-block qb, the 8 cores exchange attention features with a
single 8-core AllToAll per qb (both head-pairs staged into one [8,2,128,64]
DRAM tile; slot p carries this core's 256 feature rows for p's 64 tokens).
Every slot carries real data because row ownership is split across both batch
groups. The receiver assembles the full 1024-feature attnT for its 128 owned
rows (64 per batch) and runs the FULL out-projection locally (same FLOPs as
the partial out-proj + ReduceScatter of y-partials, but 4x less collective
traffic and half the serial CC-core occupancy).

Scheduling rules learned from traces (engine queues are strict FIFO, so any
op waiting on a slow input head-of-line-blocks its whole engine):
  - out-proj/LN for q-block c-2 (not c-1!) is emitted mid-chunk c, so its
    collective finished a full chunk earlier and none of its PE/DVE/ACT ops
    ever wait (waiting ops would stall chunk c's exp/normalize streams and
    HAM-throttle the PE to 1.2 GHz).
  - attn@V matmuls trail the score matmuls by one key-block pair so the PE
    does not wait on the exp.
  - a2a read-DMAs are emitted at chunk end; Tile assigns collective-semaphore
    waits by cumulative count, so reads must not follow later collectives.
  - softmax reciprocal uses reciprocal_approx_fast (51 ULP, fine for a
    denominator in [0.02, 2500]); the stock reciprocal is ~6 Newton passes =
    3.3us on a [1,512] single-lane tile. Input must be staged to SBUF first
    (the custom DVE op mis-reads PSUM).

Other structure vs the naive version:
  - QKV (phase A) is interleaved with attention (phase B) per 512-token chunk,
    so the scalar engine's exp stream starts ~15us earlier and PE stays warm.
  - exp is batched: scores for 2 key-blocks accumulate into one [128,1024]
    2-bank PSUM tile -> one ACT exp instruction (halves ACT instruction count).
  - causal masking is done ON the tensor engine: a -30000 mask block is
    accumulated into the diagonal scores region via an identity-stationary
    matmul (start=False), replacing per-block DVE multiplies.
  - softmax eps add is dropped (denominator >= ~1e-2, eps=1e-9 is noise).
  - layernorm: sum(z^2) comes free from an ACT Square accum_out; centering and
    inv-std scaling fuse into one two-op tensor_scalar.

Compute dtype: bf16 matmul inputs, fp32 PSUM accumulation, fp32 softmax
denominators / layernorm statistics.

Layout rule (walrus limit): DMA descriptors carry at most 2 sync waits, and
Tile's wait assignment is not transitively minimal -- a DMA touching a recycled
SBUF region inherits waits on all 8 DMA queues. So every DMA-touched SBUF tile
lives in the never-recycled top-level pool; recycled phase pools hold
compute-engine-only tiles.
"""

import sys
import numpy as np

if "/opt/trn_rl_repo" not in sys.path:
    sys.path.insert(0, "/opt/trn_rl_repo")

B, T, D, H, HD = 2, 2048, 1024, 16, 64
NCORES = 8
ALL8 = [[0, 1, 2, 3, 4, 5, 6, 7]]
HL = 4            # heads per core
FL = HL * HD      # 256 local features
RS = 512          # output token rows per core
P = 128
QB = 512          # query block
KB = 128          # key block (partition axis)
NQB = T // QB     # 4
NDCH = D // P     # 8
OWN = 64          # rows owned per (batch, qb)

TRACE = False
LAST_RESULT = None
_GRAPH = None


def _build():
    import concourse.bass as bass
    import concourse.mybir as mybir
    from concourse import bacc, tile
    from concourse.masks import make_identity

    f32 = mybir.dt.float32
    bf16 = mybir.dt.bfloat16
    AF = mybir.ActivationFunctionType
    AX = mybir.AxisListType

    nc = bacc.Bacc(num_devices=NCORES)

    xT_ext = nc.declare_dram_parameter("xT", [D, T], bf16, isOutput=False)
    wq_ext = nc.declare_dram_parameter("wq", [D, FL], bf16, isOutput=False)
    wk_ext = nc.declare_dram_parameter("wk", [D, FL], bf16, isOutput=False)
    wv_ext = nc.declare_dram_parameter("wv", [D, FL], bf16, isOutput=False)
    bq_ext = nc.declare_dram_parameter("bq", [FL, 1], f32, isOutput=False)
    bk_ext = nc.declare_dram_parameter("bk", [FL, 1], f32, isOutput=False)
    bv_ext = nc.declare_dram_parameter("bv", [1, FL], f32, isOutput=False)
    wo_ext = nc.declare_dram_parameter("wo", [D, D], bf16, isOutput=False)
    bo_ext = nc.declare_dram_parameter("bo", [1, D], f32, isOutput=False)
    g_ext = nc.declare_dram_parameter("g", [1, D], f32, isOutput=False)
    bt_ext = nc.declare_dram_parameter("bt", [1, D], f32, isOutput=False)
    xr_ext = nc.declare_dram_parameter("xr", [RS, D], bf16, isOutput=False)
    out_ext = nc.declare_dram_parameter("out", [RS, D], f32, isOutput=True)

    with tile.TileContext(nc) as tc:
        with (
            tc.tile_pool(name="res", bufs=1) as res,
            tc.tile_pool(name="dram", bufs=1, space="DRAM") as dram,
        ):
            # ============ top-level (never-recycled) tiles ============
            # --- DMA-loaded inputs; issue order = critical path first:
            # biases -> xT chunk0 (split sync/gpsimd) -> wq -> wk -> wv
            # -> xT chunks 1..3 -> wo -> xr -> broadcast rows.
            # critical path: xT chunk0 + wq feed the very first matmul group.
            # Spread them over three DMA queues (sync/vector/scalar) and keep
            # the 16 tiny bias loads from clogging the sync queue head.
            xTt = [res.tile([P, T], bf16, tag=f"xTt{d}", name=f"xTt{d}") for d in range(NDCH)]
            for dch in range(NDCH):
                engq = nc.sync if dch % 2 == 0 else nc.gpsimd
                engq.dma_start(xTt[dch][:, 0:QB], xT_ext[dch * P:(dch + 1) * P, 0:QB])
            wqb = [res.tile([P, FL], bf16, tag=f"wqb{i}", name=f"wqb{i}") for i in range(NDCH)]
            wkb = [res.tile([P, FL], bf16, tag=f"wkb{i}", name=f"wkb{i}") for i in range(NDCH)]
            wvb = [res.tile([P, FL], bf16, tag=f"wvb{i}", name=f"wvb{i}") for i in range(NDCH)]
            for dch in range(NDCH):
                nc.scalar.dma_start(wqb[dch][:], wq_ext[dch * P:(dch + 1) * P, :])
            # broadcast-source rows early (tiny; gate chunk-0 setup)
            bvr = res.tile([1, FL], f32, tag="bvr", name="bvr")
            nc.scalar.dma_start(bvr[:], bv_ext[:, :])
            # biases coalesced into one DMA each (each dma_start costs ~600ns
            # of queue time regardless of size)
            bq2 = res.tile([P, 2], f32, tag="bq2", name="bq2")
            nc.sync.dma_start(bq2[:], bq_ext[:].rearrange("(i p) o -> p (i o)", p=P))
            bk2 = res.tile([P, 2], f32, tag="bk2", name="bk2")
            nc.sync.dma_start(bk2[:], bk_ext[:].rearrange("(i p) o -> p (i o)", p=P))

            for dch in range(NDCH):
                nc.scalar.dma_start(wvb[dch][:], wv_ext[dch * P:(dch + 1) * P, :])
            for dch in range(NDCH):
                nc.sync.dma_start(wkb[dch][:], wk_ext[dch * P:(dch + 1) * P, :])
            # --- chunk-0 compute setup, ahead of the bulk DMA on gpsimd ---
            vt = [res.tile([P, HL * (HD + 1)], bf16, tag=f"vt{i}", name=f"vt{i}")
                  for i in range(T // P)]
            for tt in range(T // P):
                v3 = vt[tt][:].rearrange("p (h e) -> p h e", e=HD + 1)
                nc.gpsimd.memset(v3[:, :, HD:HD + 1], 1.0)
            eps_t = res.tile([P, 1], f32, tag="eps", name="eps")
            nc.gpsimd.memset(eps_t[:], 1e-5)
            # additive causal mask for the diagonal 128x128 block:
            # maskM[k, q] = 0 if q >= k else -30000  (exp((S-30000)/8) == 0)
            maskM = res.tile([P, KB], bf16, tag="maskM", name="maskM")
            nc.gpsimd.memset(maskM[:], 0.0)
            nc.gpsimd.affine_select(
                out=maskM[:], in_=maskM[:],
                compare_op=mybir.AluOpType.is_ge, fill=-30000.0,
                base=0, channel_multiplier=-1,
                pattern=[[1, KB]],
            )
            ident = res.tile([P, P], bf16, tag="ident", name="ident")
            make_identity(nc, ident[:])
            bias_v = res.tile([P, FL], f32, tag="bias_v", name="bias_v")
            nc.gpsimd.partition_broadcast(bias_v[:], bvr[:])
            qTz = [res.tile([P, T], bf16, tag=f"qTz{i}", name=f"qTz{i}") for i in range(4)]
            for i in range(4):
                e = i % 2
                nc.vector.memset(qTz[i][(1 - e) * HD:(2 - e) * HD, :], 0.0)
            kT = [res.tile([P, T], bf16, tag=f"kT{i}", name=f"kT{i}") for i in range(2)]

            # --- bulk loads for later chunks ---
            for tch in range(1, NQB):
                for dch in range(NDCH):
                    engq = nc.sync if dch % 2 == 0 else nc.gpsimd
                    engq.dma_start(xTt[dch][:, tch * QB:(tch + 1) * QB],
                                   xT_ext[dch * P:(dch + 1) * P, tch * QB:(tch + 1) * QB])
            # full W_out, feature-major chunks
            wob = [res.tile([P, D], bf16, tag=f"wob{i}", name=f"wob{i}") for i in range(NDCH)]
            for dch in range(NDCH):
                nc.sync.dma_start(wob[dch][:], wo_ext[dch * P:(dch + 1) * P, :])
            # residual rows (bf16 cast); row layout: (qb, batch, 64)
            xrb = [res.tile([P, D], bf16, tag=f"xrb{i}", name=f"xrb{i}") for i in range(NQB)]
            for i in range(NQB):
                nc.sync.dma_start(xrb[i][:], xr_ext[i * P:(i + 1) * P, :])
            bor = res.tile([1, D], f32, tag="bor", name="bor")
            nc.gpsimd.dma_start(bor[:], bo_ext[:, :])
            gr = res.tile([1, D], f32, tag="gr", name="gr")
            nc.gpsimd.dma_start(gr[:], g_ext[:, :])
            btr = res.tile([1, D], f32, tag="btr", name="btr")
            nc.gpsimd.dma_start(btr[:], bt_ext[:, :])

            attnT = [res.tile([P, T], bf16, tag=f"attnT{i}", name=f"attnT{i}") for i in range(2)]
            # received attention features, double-buffered per (qb parity, hp):
            # [128 feat, (4 src, 2 batch-half, 64 tok)]
            allA = [[res.tile([P, 4 * P], bf16, tag=f"allA{par}{hp}", name=f"allA{par}{hp}")
                     for hp in range(2)] for par in range(2)]
            # layernorm broadcast tiles
            bo_bc = res.tile([P, D], f32, tag="bo_bc", name="bo_bc")
            nc.gpsimd.partition_broadcast(bo_bc[:], bor[:])
            g_bc = res.tile([P, D], f32, tag="g_bc", name="g_bc")
            nc.gpsimd.partition_broadcast(g_bc[:], gr[:])
            bt_bc = res.tile([P, D], f32, tag="bt_bc", name="bt_bc")
            nc.gpsimd.partition_broadcast(bt_bc[:], btr[:])
            bo_bcb = res.tile([P, D], bf16, tag="bo_bcb", name="bo_bcb")
            nc.vector.tensor_copy(bo_bcb[:], bo_bc[:])
            xrz = [res.tile([P, D], bf16, tag=f"xrz{i}", name=f"xrz{i}") for i in range(NQB)]
            for i in range(NQB):
                nc.vector.tensor_add(xrz[i][:], xrb[i][:], bo_bcb[:])
            ont = [res.tile([P, D], f32, tag=f"on{i}", name=f"on{i}") for i in range(NQB)]

            # per-(qb,hp) AllToAll bounce buffers: slot p = [128 feat, 64 tok]
            a2a_in = [dram.tile([NCORES, 2, P, OWN], bf16, name=f"a2a_in{q}")
                      for q in range(NQB)]
            a2a_out = [dram.tile([NCORES, 2, P, OWN], bf16, name=f"a2a_out{q}")
                       for q in range(NQB)]
            # last q-block uses contiguous per-head-pair tiles so its exchange
            # splits into two collectives (h0's overlaps h1's attention)
            a2a_in3 = [dram.tile([NCORES, P, OWN], bf16, name=f"a2a_in3h{h}")
                       for h in range(2)]
            a2a_out3 = [dram.tile([NCORES, P, OWN], bf16, name=f"a2a_out3h{h}")
                        for h in range(2)]

            with (
                tc.tile_pool(name="psA", bufs=2, space="PSUM") as psA,
                tc.tile_pool(name="psS", bufs=2, space="PSUM") as psS,
                tc.tile_pool(name="psAt", bufs=1, space="PSUM") as psAt,
                tc.tile_pool(name="Ep", bufs=4) as Ep,
                tc.tile_pool(name="small", bufs=2) as small,
                tc.tile_pool(name="zp", bufs=2) as zp,
            ):
                def emit_qk(tch, ft):
                    # q^T, k^T for one head-pair (ft); attention for hp==ft
                    # depends only on this half, so it can start while the
                    # other half's projections run
                    for wb, is_q, bias in ((wqb, True, bq2), (wkb, False, bk2)):
                        if True:
                            ps = psA.tile([P, QB], f32, tag="psqk", name="psqk")
                            for dch in range(NDCH):
                                nc.tensor.matmul(ps[:], wb[dch][:, ft * P:(ft + 1) * P],
                                                 xTt[dch][:, tch * QB:(tch + 1) * QB],
                                                 start=(dch == 0), stop=(dch == NDCH - 1))
                            if is_q:
                                for e in range(2):
                                    nc.vector.tensor_scalar_add(
                                        qTz[ft * 2 + e][e * HD:(e + 1) * HD,
                                                        tch * QB:(tch + 1) * QB],
                                        ps[e * HD:(e + 1) * HD, :],
                                        bias[e * HD:(e + 1) * HD, ft:ft + 1])
                            else:
                                nc.vector.tensor_scalar_add(
                                    kT[ft][:, tch * QB:(tch + 1) * QB], ps[:],
                                    bias[:, ft:ft + 1])
                def emit_v(tch):
                    # v (token-major) + bias
                    for i in range(QB // P):
                        tt = tch * 4 + i
                        psv = psA.tile([P, QB], f32, tag="psqk", name="psv")
                        for dch in range(NDCH):
                            nc.tensor.matmul(psv[:, 0:FL], xTt[dch][:, tt * P:(tt + 1) * P],
                                             wvb[dch][:],
                                             start=(dch == 0), stop=(dch == NDCH - 1))
                        v3 = vt[tt][:].rearrange("p (h e) -> p h e", e=HD + 1)
                        nc.vector.tensor_tensor(
                            v3[:, :, 0:HD],
                            psv[:, 0:FL].rearrange("p (h d) -> p h d", d=HD),
                            bias_v[:].rearrange("p (h d) -> p h d", d=HD),
                            op=mybir.AluOpType.add)

                def emit_attn_hp(qb, hp, vhook=None):
                    nkb = (qb + 1) * (QB // KB)
                    if True:
                        pa = [psAt.tile([HD + 1, QB], f32, tag=f"psa{e}", name=f"psa{e}")
                              for e in range(2)]

                        def emit_av(i, Et):
                            # attn@V for pair i, consuming its exp'd Et tiles
                            for half in range(2):
                                kb = 2 * i + half
                                lo = max(0, (kb - qb * (QB // KB)) * KB)
                                mlo = 0 if kb == 0 else lo
                                base = half * QB
                                v3 = vt[kb][:].rearrange("p (h e) -> p h e", e=HD + 1)
                                for e in range(2):
                                    nc.tensor.matmul(
                                        pa[e][:, mlo:QB], v3[:, hp * 2 + e, :],
                                        Et[e][:, base + mlo:base + QB],
                                        start=(kb == 0), stop=(kb == nkb - 1))

                        prev = None
                        for i in range(nkb // 2):
                            # pair of key blocks (2i, 2i+1) -> one 2-bank PSUM
                            # tile + one exp per head e. Both halves' score
                            # matmuls span [plo:QB] so the exp window is fully
                            # written (masked cols are excluded from attn@V by
                            # the per-block window instead). attn@V trails one
                            # pair behind the scores so the PE never waits on
                            # the exp.
                            plo = max(0, (2 * i - qb * (QB // KB)) * KB)
                            ps = [psS.tile([P, 2 * QB], f32, tag="pss", name="pss")
                                  for _ in range(2)]
                            Et = [Ep.tile([P, 2 * QB], bf16, tag="E", name="E")
                                  for _ in range(2)]
                            for half in range(2):
                                kb = 2 * i + half
                                diag = kb - qb * (QB // KB) >= 0
                                base = half * QB
                                for e in range(2):
                                    nc.tensor.matmul(
                                        ps[e][:, base + plo:base + QB],
                                        kT[hp][:, kb * KB:(kb + 1) * KB],
                                        qTz[hp * 2 + e][:, qb * QB + plo:(qb + 1) * QB],
                                        start=True, stop=not diag)
                                if diag:
                                    lo = (kb - qb * (QB // KB)) * KB
                                    for e in range(2):
                                        nc.tensor.matmul(
                                            ps[e][:, base + lo:base + lo + KB],
                                            ident[:], maskM[:],
                                            start=False, stop=True)
                            for e in range(2):
                                nc.scalar.activation(Et[e][:, plo:2 * QB],
                                                     ps[e][:, plo:2 * QB],
                                                     AF.Exp, scale=0.125)
                            if i == 1 and vhook is not None:
                                # slot this chunk's v-projection behind the
                                # first score pairs so exp starts immediately
                                # at the chunk boundary
                                vhook()
                            if prev is not None:
                                emit_av(*prev)
                            prev = (i, Et)
                        emit_av(*prev)
                        for e in range(2):
                            den = small.tile([1, QB], f32, tag="den", name="den")
                            nc.vector.tensor_copy(den[:], pa[e][HD:HD + 1, :])
                            rec = small.tile([1, QB], f32, tag="rec", name="rec")
                            nc.vector.reciprocal_approx_fast(out=rec[:], in_=den[:])
                            bc = small.tile([HD, QB], f32, tag="bc", name="bc")
                            nc.gpsimd.partition_broadcast(bc[:], rec[:])
                            nc.vector.tensor_tensor(
                                attnT[hp][e * HD:(e + 1) * HD, qb * QB:(qb + 1) * QB],
                                pa[e][0:HD, :], bc[:], op=mybir.AluOpType.mult)
                        # ship this (qb, hp)'s features: slot p = my 128 feature
                        # rows for p's 64 tokens; one collective per qb after
                        # both head-pairs are staged
                        srcv = attnT[hp][:, qb * QB:(qb + 1) * QB].rearrange(
                            "p (s t) -> p s t", t=OWN)
                        if qb == NQB - 1:
                            dst = a2a_in3[hp][:].rearrange("s p t -> p s t")
                            nc.sync.dma_start(dst, srcv)
                            nc.gpsimd.collective_compute(
                                "AllToAll", mybir.AluOpType.bypass,
                                replica_groups=ALL8,
                                ins=[a2a_in3[hp][:].opt()],
                                outs=[a2a_out3[hp][:].opt()],
                            )
                        else:
                            dst = a2a_in[qb][:, hp].rearrange("s p t -> p s t")
                            nc.sync.dma_start(dst, srcv)
                            if hp == 1:
                                nc.gpsimd.collective_compute(
                                    "AllToAll", mybir.AluOpType.bypass,
                                    replica_groups=ALL8,
                                    ins=[a2a_in[qb][:].opt()],
                                    outs=[a2a_out[qb][:].opt()],
                                )

                def emit_a2a_read(qb, hps=(0, 1)):
                    aA = allA[qb % 2]
                    last = qb == NQB - 1
                    for hp in hps:
                        for bh in range(2):
                            if last:
                                src = a2a_out3[hp][bh * 4:(bh + 1) * 4].rearrange(
                                    "s p t -> p s t")
                            else:
                                src = a2a_out[qb][bh * 4:(bh + 1) * 4, hp].rearrange(
                                    "s p t -> p s t")
                            dstv = aA[hp][:].rearrange(
                                "p (s q2 t) -> p s q2 t", q2=2, t=OWN)[:, :, bh, :]
                            nc.sync.dma_start(dstv, src)

                def emit_out(qb):
                    aA = allA[qb % 2]
                    psy = psS.tile([P, 2 * QB], f32, tag="pss", name="psy")
                    # dt-major: bank 0 completes after 8 matmuls, so its
                    # residual-add runs under bank 1's matmuls
                    z = zp.tile([P, D], f32, tag="z", name="z")
                    for dt in range(2):
                        # hp0 feature chunks first: on the last q-block they
                        # can run while hp1's exchange is still in flight
                        for n, fc in enumerate((0, 2, 4, 6, 1, 3, 5, 7)):
                            s, hp = fc // 2, fc % 2
                            nc.tensor.matmul(
                                psy[:, dt * QB:(dt + 1) * QB],
                                aA[hp][:, s * P:(s + 1) * P],
                                wob[fc][:, dt * QB:(dt + 1) * QB],
                                start=(n == 0), stop=(n == NDCH - 1))
                        nc.vector.tensor_tensor(
                            z[:, dt * QB:(dt + 1) * QB], psy[:, dt * QB:(dt + 1) * QB],
                            xrz[qb][:, dt * QB:(dt + 1) * QB], op=mybir.AluOpType.add)
                    s1 = small.tile([P, 1], f32, tag="s1", name="s1")
                    nc.vector.reduce_sum(s1[:], z[:], axis=AX.X)
                    mu = small.tile([P, 1], f32, tag="mu", name="mu")
                    nc.vector.tensor_scalar_mul(mu[:], s1[:], 1.0 / D)
                    # sum(z^2)/D via ACT Square accumulator (scale folded: (z/sqrt(D))^2)
                    scr = zp.tile([P, D], f32, tag="scr", name="scr")
                    zsq = small.tile([P, 1], f32, tag="zsq", name="zsq")
                    nc.scalar.activation(scr[:], z[:], AF.Square,
                                         scale=float(1.0 / np.sqrt(D)), accum_out=zsq[:])
                    mu2 = small.tile([P, 1], f32, tag="mu2", name="mu2")
                    nc.gpsimd.tensor_tensor(mu2[:], mu[:], mu[:], op=mybir.AluOpType.mult)
                    var = small.tile([P, 1], f32, tag="var", name="var")
                    nc.gpsimd.tensor_tensor(var[:], zsq[:], mu2[:],
                                            op=mybir.AluOpType.subtract)
                    # rsqrt(var + eps) = exp(-0.5 * ln(var + eps)): Log and Exp
                    # share one ACT table set, so no table switch away from the
                    # attention exps (Sqrt lives in a different set and cost
                    # ~2.6us of table reloads per LN)
                    lnv = small.tile([P, 1], f32, tag="lnv", name="lnv")
                    nc.scalar.activation(lnv[:], var[:], AF.Ln, bias=eps_t[:])
                    ri = small.tile([P, 1], f32, tag="ri", name="ri")
                    nc.scalar.activation(ri[:], lnv[:], AF.Exp, scale=-0.5)
                    on = ont[qb]
                    nc.vector.tensor_scalar(on[:], z[:], mu[:], ri[:],
                                            op0=mybir.AluOpType.subtract,
                                            op1=mybir.AluOpType.mult)
                    nc.vector.tensor_mul(on[:], on[:], g_bc[:])
                    nc.vector.tensor_add(on[:], on[:], bt_bc[:])
                    nc.sync.dma_start(out_ext[qb * P:(qb + 1) * P, :], on[:])

                for c in range(NQB):
                    emit_qk(c, 0)
                    emit_v(c)
                    emit_attn_hp(c, 0)
                    emit_qk(c, 1)
                    if c >= 2:
                        emit_out(c - 2)
                    if c == NQB - 1:
                        # h0's exchange completes during h1's attention; fetch
                        # it now (before h1's collective, for a precise wait)
                        # so out(last)'s hp0 matmuls run under h1's exchange
                        emit_a2a_read(c, hps=(0,))
                        emit_attn_hp(c, 1)
                        emit_a2a_read(c, hps=(1,))
                    else:
                        emit_attn_hp(c, 1)
                        emit_a2a_read(c)
                emit_out(NQB - 2)
                emit_out(NQB - 1)

    nc.finalize()
    return nc


def kernel(**inputs):
    global _GRAPH, LAST_RESULT
    import ml_dtypes
    from concourse.bass_utils import run_bass_kernel_spmd
    bfnp = ml_dtypes.bfloat16

    x = np.ascontiguousarray(inputs["x"], dtype=np.float32)
    W_proj = np.ascontiguousarray(inputs["W_proj"], dtype=np.float32)
    b_proj = np.ascontiguousarray(inputs["b_proj"], dtype=np.float32)
    W_out = np.ascontiguousarray(inputs["W_out"], dtype=np.float32)
    b_out = np.ascontiguousarray(inputs["b_out"], dtype=np.float32)
    ln_g = np.ascontiguousarray(inputs["ln_gamma"], dtype=np.float32)
    ln_b = np.ascontiguousarray(inputs["ln_beta"], dtype=np.float32)

    if _GRAPH is None:
        _GRAPH = _build()
    nc = _GRAPH

    wo_full = np.ascontiguousarray(W_out).astype(bfnp)
    in_maps = []
    for c in range(NCORES):
        b, r = c // 4, c % 4
        cs = slice(r * FL, (r + 1) * FL)
        xr_rows = np.concatenate(
            [x[bb, qq * QB + c * OWN: qq * QB + (c + 1) * OWN]
             for qq in range(NQB) for bb in range(B)], axis=0)
        in_maps.append({
            "xT": np.ascontiguousarray(x[b].T).astype(bfnp),
            "wq": np.ascontiguousarray(W_proj[:, cs]).astype(bfnp),
            "wk": np.ascontiguousarray(W_proj[:, D + r * FL: D + (r + 1) * FL]).astype(bfnp),
            "wv": np.ascontiguousarray(W_proj[:, 2 * D + r * FL: 2 * D + (r + 1) * FL]).astype(bfnp),
            "bq": np.ascontiguousarray(b_proj[cs].reshape(FL, 1)),
            "bk": np.ascontiguousarray(b_proj[D + r * FL: D + (r + 1) * FL].reshape(FL, 1)),
            "bv": np.ascontiguousarray(b_proj[2 * D + r * FL: 2 * D + (r + 1) * FL].reshape(1, FL)),
            "wo": wo_full,
            "bo": b_out.reshape(1, D),
            "g": ln_g.reshape(1, D),
            "bt": ln_b.reshape(1, D),
            "xr": np.ascontiguousarray(xr_rows).astype(bfnp),
        })

    res = run_bass_kernel_spmd(nc, in_maps, core_ids=list(range(NCORES)),
                               trace=TRACE)
    LAST_RESULT = res

    out = np.empty((B, T, D), np.float32)
    for c in range(NCORES):
        oc = res.results[c]["out"]
        for qq in range(NQB):
            for bb in range(B):
                out[bb, qq * QB + c * OWN: qq * QB + (c + 1) * OWN, :] = \
                    oc[qq * P + bb * OWN: qq * P + (bb + 1) * OWN]
    return out
